# revision 26
# baseline (speedup 1.0000x reference)
"""Trainium2 Bass kernel for nn_CapsuleLayer (capsule dynamic routing).

Math (reference):
    u_hat[b,c,u,s] = sum_i W[c,u,s,i] * x[b,i,c]          (never materialized here)
    3 routing iterations:
        c_ij = softmax_u(b_ij)                            [C, U]
        s_j[b,u,s]  = sum_c c_ij[c,u] * u_hat[b,c,u,s]
        v_j = squash(s_j)   (norm over the U axis!)
        u_vj1[c,u] = sum_{b,s} u_hat[b,c,u,s] v_j[b,u,s] / B
        b_ij += u_vj1
    output = v_j  (B, U, S, 1)

Sharding: channels C=1152 split 8 ways (CL=144 per core).  Per core both
x-slice and W-slice live in SBUF, so u_hat is recomputed on the fly as
matrix products (contraction over (c,i)=2304 or over b=256), all shapes
128-partition friendly.  The only cross-core data dependency is the
s_j partial sum: one 160 KB f16 AllReduce per routing iteration.

v3 engine/latency notes:
  - collective staging DMAs ride the SP/HWDGE queue (gpsimd SWDGE adds
    ~1us descriptor-gen per hop + long sem gaps);
  - squash runs entirely on DVE (Newton rsqrt; keeping sqrt off the
    Activation engine avoids act-table swaps against exp);
  - the routing reduce over s is folded into S=32 accumulating
    "em" matmuls on PE (the DVE was the routing-phase tail otherwise);
  - dummy warm-up matmuls gated on the AllReduce result keep the PE
    p-state high through each collective gap (0.65/1.2/2.4 GHz ramp).

v4 notes (timeline-sim guided):
  - xf load deferred past iteration-0 MM1: that MM1 is HBM-bound, and
    xf (1.2 MB, first needed by the m-phase ~40us later) was stealing
    bandwidth right before AllReduce #1; it now streams during the AR.
  - b_ij update + softmax split per chunk-half so bm group 0 and the
    next MM1's first chunk group start during the selector-h1 tail
    (was a ~2.6us PE bubble per routing iteration).
  - final 16-row squash fused to one op chain (per-op overhead
    dominates at PR=16 rows).
  - wm loads ride the Act HWDGE queue, xt/em the SP queue: the two
    queues issue concurrently at load time, so iteration-0 delivery
    approaches the HBM floor instead of queue-serializing.
  - dead ends, measured: f8e4 AllReduce wire works mechanically but
    CCE requantizes per ring hop -> rel_err 0.036 > 2e-2 gate;
    cc_dim="Free" ReduceScatter interleaves shards fine-grained (not
    contiguous 80-col blocks), breaking the on-core squash u-grouping;
    remote_dma/remote_dma_broadcast (P2P SBUF exchange, would replace
    the ~30us collectives with ~5us) is rejected by this runtime --
    execution fails and wedges the core even for a self-send.

Per-core layouts (host-prepared):
    XT [128, T*B ] f16 : XT[p, t*256+b]      = x[b, i, c],  ci = 128t+p
    XF [128, 2*KCI] f16: XF[p, bc*2304+ci]   = x[b, i, c],  b  = 128bc+p
    WM [128, T*US] f16 : WM[p, t*320+s*10+u] = W[c, u, s, i], ci = 128t+p
    EM [128, 128] f32   : block-diag selector, EM[p,m] = (p//16==m//16)/256
Column convention for the (u,s) axis everywhere: col = s*10 + u.
"""

import numpy as np

B, IN_U, C, NUM_U, S = 256, 16, 1152, 10, 32
NCORES = 8
CL = C // NCORES          # 144 channels per core
KCI = CL * IN_U           # 2304 contraction size
T = KCI // 128            # 18 partition chunks
US = NUM_U * S            # 320
NITER = 3
G = 3                     # chunk groups (pipelining granularity)
CPG = T // G              # 6 chunks per group
RSQRT_MAGIC = 0x5F3759DF
NWARM_SQ = 2              # warm-up matmuls racing the squash chain
NWARM_SM = 4              # warm-up matmuls racing softmax+bm

_CACHE = {}


def _build_program(bypass_cc=False, reps=1, shared_cc=False, warm=True,
                   f8_ar=False):
    import concourse.bacc as bacc
    import concourse.tile as tile
    from concourse import mybir
    from contextlib import ExitStack

    f32 = mybir.dt.float32
    f16 = mybir.dt.float16
    f8 = mybir.dt.float8e4
    i32 = mybir.dt.int32
    ar_dt = f8 if f8_ar else f16
    AX = mybir.AxisListType
    ALU = mybir.AluOpType
    AF = mybir.ActivationFunctionType

    # Bacc (not raw Bass): its compile() pipeline legalizes multi-wait
    # instructions (move_matmul_waits_to_ldweights + generate_event_semaphores)
    # which walrus codegen otherwise rejects ("Too many sync wait commands").
    nc = bacc.Bacc(None, num_devices=NCORES)
    xt_d = nc.declare_dram_parameter("xt", [128, T * B], f16, isOutput=False)
    xf_d = nc.declare_dram_parameter("xf", [128, 2 * KCI], f16, isOutput=False)
    wm_d = nc.declare_dram_parameter("wm", [128, T * US], f16, isOutput=False)
    em_d = nc.declare_dram_parameter("em", [128, 128], f16, isOutput=False)
    out_d = nc.declare_dram_parameter("out", [128, 2 * US], f32, isOutput=True)

    with tile.TileContext(nc) as tc, ExitStack() as ctx:
        singles = ctx.enter_context(tc.tile_pool(name="singles", bufs=1))
        big = ctx.enter_context(tc.tile_pool(name="big", bufs=1))
        work = ctx.enter_context(tc.tile_pool(name="work", bufs=2))
        qbig = ctx.enter_context(tc.tile_pool(name="qbig", bufs=2))
        psum_s = ctx.enter_context(tc.tile_pool(name="psum_s", bufs=1, space="PSUM"))
        psum_m = ctx.enter_context(tc.tile_pool(name="psum_m", bufs=3, space="PSUM"))
        psum_u = ctx.enter_context(tc.tile_pool(name="psum_u", bufs=1, space="PSUM"))
        psum_w = ctx.enter_context(tc.tile_pool(name="psum_w", bufs=1, space="PSUM"))
        dram = ctx.enter_context(tc.tile_pool(name="dram", bufs=2, space="DRAM"))

        for _rep in range(reps):
            # Loads: em first (it feeds the warm-up matmuls), then xt/wm
            # in fine-grained slices so iteration-0 MM1 starts early.
            em_sb = singles.tile([128, 128], f16, name="em_sb")
            nc.sync.dma_start(out=em_sb, in_=em_d[:])
            # DMA-free warm operand: the initial PE warm chain starts at
            # ~0.7us (memset latency) instead of waiting for the em tile
            zw_sb = singles.tile([128, 128], f16, name="zw_sb")
            nc.vector.memset(zw_sb, 0.0)
            xt_sb = singles.tile([128, T * B], f16, name="xt_sb")
            wm_sb = singles.tile([128, T * US], f16, name="wm_sb")
            for g2 in range(2 * G):
                h = CPG // 2
                nc.sync.dma_start(
                    out=xt_sb[:, g2 * h * B : (g2 + 1) * h * B],
                    in_=xt_d[:, g2 * h * B : (g2 + 1) * h * B],
                )
                # wm rides the Act-engine HWDGE queue: two queues issue
                # concurrently at load time (Act is idle), so delivery
                # approaches the HBM floor instead of queue-serializing
                nc.scalar.dma_start(
                    out=wm_sb[:, g2 * h * US : (g2 + 1) * h * US],
                    in_=wm_d[:, g2 * h * US : (g2 + 1) * h * US],
                )
            # xf (1.2 MB) feeds only the routing m-phase; loading it here
            # would steal HBM bandwidth from the xt/wm stream that gates
            # iteration-0 MM1 (which is HBM-bound).  Deferred below so it
            # rides the DMA queue during the first AllReduce's dead time.
            xf_sb = singles.tile([128, 2 * KCI], f16, name="xf_sb")
            bij_sb = singles.tile([128, T * NUM_U], f32, name="bij_sb")
            magic_sb = singles.tile([128, 1], i32, name="magic_sb")
            nc.vector.memset(magic_sb, RSQRT_MAGIC)
            # dummy act op emitted after the wm DMAs: pulls the ~1.3us
            # LoadActFuncSet off the Act queue head (where it delayed wm
            # slice 0 and hence MM1's first chunk) into idle Act time
            warmact_sb = singles.tile([128, 1], f32, name="warmact_sb")
            nc.scalar.activation(
                out=warmact_sb, in_=magic_sb.bitcast(f32), func=AF.Exp
            )

            def pe_warm(n, gate=None, cols=512):
                """n dummy matmuls to hold/raise the PE p-state.  `gate`
                ties the first one to a data dependency (e.g. the AllReduce
                result) so they fire exactly when the real gap starts."""
                if not warm:
                    return
                # one PSUM accumulation group: consecutive matmuls flow
                # back-to-back with no semaphore hops, so the p-state ramp
                # actually accumulates (separate groups re-enter LOW each
                # time via the WAW sem chain)
                ps = psum_w.tile([128, 512], f32, name="warm_ps")
                for k in range(n):
                    if gate is not None and k == 0:
                        rhs = gate[:, 0:cols]
                    elif cols <= 128:
                        rhs = zw_sb
                    else:
                        rhs = xt_sb[:, 0:cols]
                    nc.tensor.matmul(
                        ps[:, 0:cols],
                        lhsT=zw_sb if cols <= 128 else em_sb,
                        rhs=rhs,
                        start=(k == 0),
                        stop=(k == n - 1),
                    )

            def mm1(rhs_groups, scale, dt=f16):
                """s_partial[b,(s,u)] = XT.T @ rhs, scaled; -> [128, 2*US]."""
                cc_sb = work.tile([128, 2 * US], dt, name="cc_sb")
                ps = [psum_s.tile([128, US], f32, name=f"s_ps{bc}") for bc in range(2)]
                for t in range(T):
                    rhs = rhs_groups[t // CPG]
                    tl = t % CPG
                    for bc in range(2):
                        nc.tensor.matmul(
                            ps[bc],
                            lhsT=xt_sb[:, t * B + bc * 128 : t * B + bc * 128 + 128],
                            rhs=rhs[:, tl * US : (tl + 1) * US],
                            start=(t == 0),
                            stop=(t == T - 1),
                        )
                nc.scalar.activation(
                    out=cc_sb[:, 0:US], in_=ps[0], func=AF.Copy,
                    scale=float(scale),
                )
                if scale == 1.0:
                    nc.vector.tensor_copy(out=cc_sb[:, US : 2 * US], in_=ps[1])
                else:
                    nc.vector.tensor_scalar_mul(
                        cc_sb[:, US : 2 * US], ps[1], float(scale)
                    )
                return cc_sb

            def allreduce(cc_sb, dt=f16):
                cc_in = dram.tile([128, 2 * US], dt, name="cc_in")
                cc_out = dram.tile(
                    [128, 2 * US], dt, name="cc_out",
                    addr_space="Shared" if shared_cc else "Local",
                )
                nc.sync.dma_start(out=cc_in, in_=cc_sb)
                if bypass_cc:
                    nc.sync.dma_start(out=cc_out, in_=cc_in)
                else:
                    nc.gpsimd.collective_compute(
                        "AllReduce",
                        ALU.add,
                        replica_groups=[list(range(NCORES))],
                        ins=[cc_in.opt()],
                        outs=[cc_out.opt()],
                    )
                if dt is f16:
                    s_sb = work.tile([128, 2 * US], f16, name="s_sb")
                    nc.sync.dma_start(out=s_sb, in_=cc_out)
                    return s_sb
                # f8 wire format: readback then upcast once on DVE so the
                # squash chain stays f16
                s8_sb = work.tile([128, 2 * US], dt, name="s8_sb")
                nc.sync.dma_start(out=s8_sb, in_=cc_out)
                s_sb = work.tile([128, 2 * US], f16, name="s_sb")
                nc.vector.tensor_copy(out=s_sb, in_=s8_sb)
                return s_sb

            PR = 128 // NCORES  # 16 partition rows per core after ReduceScatter

            def reduce_scatter(cc_sb):
                """Final iteration: each core only needs its 1/8 of s_j.
                (cc_dim="Free" would keep all DVE lanes busy in the tail
                squash, but its shard interleaving is fine-grained, not
                contiguous blocks — measured empirically — so the on-core
                squash would span wrong u-groups.)"""
                cc_in = dram.tile([128, 2 * US], f16, name="rs_in")
                cc_out = dram.tile([PR, 2 * US], f16, name="rs_out")
                nc.sync.dma_start(out=cc_in, in_=cc_sb)
                if bypass_cc:
                    nc.sync.dma_start(out=cc_out, in_=cc_in[0:PR, :])
                else:
                    nc.gpsimd.collective_compute(
                        "ReduceScatter",
                        ALU.add,
                        replica_groups=[list(range(NCORES))],
                        ins=[cc_in.opt()],
                        outs=[cc_out.opt()],
                    )
                s16 = work.tile([PR, 2 * US], f16, name="s16")
                nc.sync.dma_start(out=s16, in_=cc_out)
                return s16

            def squash(s_sb, out_dtype, rows=128, fused=False, width=2 * US):
                """v = s * mag/(1+mag^2); mag^2 = sum_u s^2 per (b, s').

                All DVE: square+reduce, then rsqrt via bit-trick seed + one
                Newton step (keeps sqrt off the Activation engine so the
                exp act-table never needs swapping out).  Runs per
                batch-half so the routing m-phase can consume v[:, bc0]
                while the bc1 chain is still in flight; `fused` collapses
                both halves into one op chain (final 16-row squash is all
                per-op overhead, no consumer pipelining to preserve).
                """
                v_sb = work.tile([rows, width], out_dtype, name="v_sb")
                parts = [(0, width)] if fused else [(0, US), (US, 2 * US)]
                for bc, (lo, hi) in enumerate(parts):
                    sl = slice(lo, hi)
                    ns = (hi - lo) // NUM_U  # s'-groups in this span
                    sfx = f"_{bc}"

                    def t_(name):
                        return work.tile([rows, ns], f32, name=name + sfx)

                    sq = work.tile([rows, hi - lo], f16, name="sq" + sfx)
                    nc.vector.tensor_mul(out=sq, in0=s_sb[:, sl], in1=s_sb[:, sl])
                    magsq = t_("magsq")
                    nc.vector.reduce_sum(
                        out=magsq,
                        in_=sq.rearrange("p (s u) -> p s u", s=ns),
                        axis=AX.X,
                    )
                    den = t_("den")
                    nc.vector.tensor_scalar_add(den, magsq, 1.0)
                    rden = t_("rden")
                    nc.vector.reciprocal(rden, den)
                    # rsqrt(magsq): fp32 bit-trick seed + 1 Newton step
                    yb = work.tile([rows, ns], i32, name="yb" + sfx)
                    nc.vector.tensor_scalar(
                        yb, magsq.bitcast(i32), 1, None,
                        op0=ALU.arith_shift_right,
                    )
                    nc.vector.tensor_tensor(
                        out=yb,
                        in0=magic_sb[0:rows, :].broadcast_to([rows, ns]),
                        in1=yb,
                        op=ALU.subtract,
                    )
                    y = yb.bitcast(f32)
                    tmp = t_("tmp")
                    half = t_("half")
                    nc.vector.tensor_mul(out=tmp, in0=y, in1=y)
                    nc.vector.tensor_mul(out=tmp, in0=tmp, in1=magsq)
                    nc.vector.tensor_scalar(
                        half, tmp, -0.5, 1.5, op0=ALU.mult, op1=ALU.add
                    )
                    nc.vector.tensor_mul(out=y, in0=y, in1=half)
                    fct = t_("fct")
                    nc.vector.tensor_mul(out=fct, in0=magsq, in1=y)
                    nc.vector.tensor_mul(out=fct, in0=fct, in1=rden)
                    nc.vector.tensor_mul(
                        out=v_sb[:, sl].rearrange("p (s u) -> p s u", s=ns),
                        in0=s_sb[:, sl].rearrange("p (s u) -> p s u", s=ns),
                        in1=fct[:].unsqueeze(2).broadcast_to([rows, ns, NUM_U]),
                    )
                return v_sb

            def routing_update(v_bf, first):
                """u_vj1 -> b_ij update -> softmax; returns c_ij (f16).

                Per chunk: PE accumulates m = XF.T @ v into PSUM, DVE
                multiplies W against the PSUM bank directly into q_all.
                The s-reduction + i-reduction + /B all fold into S=32
                accumulating matmuls against the block-diag EM selector.
                """
                q_all = qbig.tile([128, T * US], f16, name="q_all")
                for t in range(T):
                    ps = psum_m.tile([128, US], f32, name="m_ps")
                    for bc in range(2):
                        nc.tensor.matmul(
                            ps,
                            lhsT=xf_sb[
                                :, bc * KCI + t * 128 : bc * KCI + (t + 1) * 128
                            ],
                            rhs=v_bf[:, bc * US : (bc + 1) * US],
                            start=(bc == 0),
                            stop=(bc == 1),
                        )
                    if t % 3 == 0:
                        # direct: DVE reads the PSUM bank (f32, 1 elem/cyc)
                        nc.vector.tensor_mul(
                            out=q_all[:, t * US : (t + 1) * US],
                            in0=wm_sb[:, t * US : (t + 1) * US],
                            in1=ps,
                        )
                    else:
                        # bounced: idle Act engine copies PSUM->SBUF f16,
                        # DVE multiplies f16*f16 at 2 elem/cyc -- balances
                        # the two engines at ~5.4us each across 18 chunks
                        m_t = work.tile([128, US], f16, name="m_t", bufs=4)
                        nc.scalar.copy(out=m_t, in_=ps)
                        nc.vector.tensor_mul(
                            out=q_all[:, t * US : (t + 1) * US],
                            in0=wm_sb[:, t * US : (t + 1) * US],
                            in1=m_t,
                        )
                # two chunk-half accumulators: the first 32 matmuls only
                # need q chunks 0..8, so they run inside the m-phase's
                # DVE tail instead of after it
                TH = T // 2
                qv = q_all.rearrange("p (t s u) -> p t s u", t=T, s=S)
                ups_h = [
                    psum_u.tile([128, TH * NUM_U], f32, name=f"u_ps{h}")
                    for h in range(2)
                ]
                for h in range(2):
                    for k in range(S):
                        nc.tensor.matmul(
                            ups_h[h],
                            lhsT=em_sb,
                            rhs=qv[:, h * TH : (h + 1) * TH, k, :],
                            start=(k == 0),
                            stop=(k == S - 1),
                        )
                # per-half b_ij update + softmax: the half-0 chain (chunks
                # 0..TH-1) completes while the half-1 selector matmuls are
                # still on PE, so bm group 0 -> mm1 group 0 can start inside
                # the selector tail instead of after the full softmax.
                cij_sb = work.tile([128, T * NUM_U], f16, name="cij_sb")
                for h in range(2):
                    sl = slice(h * TH * NUM_U, (h + 1) * TH * NUM_U)
                    if first:
                        nc.vector.tensor_copy(out=bij_sb[:, sl], in_=ups_h[h])
                    else:
                        nc.vector.tensor_add(
                            out=bij_sb[:, sl], in0=bij_sb[:, sl], in1=ups_h[h]
                        )
                    # softmax over u (inner groups of 10); b_ij stays
                    # O(1)-ish (mean-over-batch agreement), so no max
                    # subtraction needed
                    ex = work.tile([128, TH * NUM_U], f32, name=f"ex{h}")
                    nc.scalar.activation(out=ex, in_=bij_sb[:, sl], func=AF.Exp)
                    sm = work.tile([128, TH], f32, name=f"sm{h}")
                    nc.vector.reduce_sum(
                        out=sm,
                        in_=ex.rearrange("p (t u) -> p t u", t=TH),
                        axis=AX.X,
                    )
                    rsm = work.tile([128, TH], f32, name=f"rsm{h}")
                    nc.vector.reciprocal(rsm, sm)
                    nc.vector.tensor_mul(
                        out=cij_sb[:, sl].rearrange("p (t u) -> p t u", t=TH),
                        in0=ex.rearrange("p (t u) -> p t u", t=TH),
                        in1=rsm[:].unsqueeze(2).broadcast_to([128, TH, NUM_U]),
                    )
                return cij_sb

            def bm_build(cij_sb):
                groups = []
                for g in range(G):
                    bm_g = big.tile([128, CPG * US], f16, name=f"bm_g{g}")
                    # first group in two halves so mm1 can start sooner
                    pieces = 2 if g == 0 else 1
                    cp = CPG // pieces
                    for pc in range(pieces):
                        t0 = g * CPG + pc * cp
                        nc.vector.tensor_mul(
                            out=bm_g[:, pc * cp * US : (pc + 1) * cp * US]
                            .rearrange("p (t s u) -> p t s u", t=cp, s=S),
                            in0=wm_sb[:, t0 * US : (t0 + cp) * US].rearrange(
                                "p (t s u) -> p t s u", t=cp, s=S
                            ),
                            in1=cij_sb[:, t0 * NUM_U : (t0 + cp) * NUM_U]
                            .rearrange("p (t u) -> p t u", t=cp)
                            .unsqueeze(2)
                            .broadcast_to([128, cp, S, NUM_U]),
                        )
                    groups.append(bm_g)
                return groups

            wm_groups = [
                wm_sb[:, g * CPG * US : (g + 1) * CPG * US] for g in range(G)
            ]
            # em-only rhs: the warm chain starts as soon as the 32 KB em
            # tile lands instead of waiting on the first 256 xt columns
            pe_warm(12, cols=128)
            v_bf = None
            for it in range(NITER):
                # the two routing ARs ride an f8 wire (payload halved; the
                # quantization only perturbs c_ij, not the output path);
                # the final ReduceScatter stays f16
                it_dt = ar_dt if it < NITER - 1 else f16
                if it == 0:
                    cc = mm1(wm_groups, 1.0 / NUM_U, dt=it_dt)
                    # deferred xf load: emitted after MM1 so it sits behind
                    # the cc staging DMA in the HWDGE queue (not gated on
                    # the collective) and streams during AllReduce #1
                    for bc in range(2):
                        nc.sync.dma_start(
                            out=xf_sb[:, bc * KCI : (bc + 1) * KCI],
                            in_=xf_d[:, bc * KCI : (bc + 1) * KCI],
                        )
                else:
                    cij = routing_update(v_bf, first=(it == 1))
                    pe_warm(NWARM_SM)
                    cc = mm1(bm_build(cij), 1.0, dt=it_dt)
                if it < NITER - 1:
                    s_sb = allreduce(cc, dt=it_dt)
                    pe_warm(NWARM_SQ, gate=s_sb)
                    v_bf = squash(s_sb, f16)
                else:
                    s16 = reduce_scatter(cc)
                    v16 = squash(s16, f32, rows=PR, fused=True)
            nc.sync.dma_start(out=out_d[0:PR, :], in_=v16)

    return nc


def _prep_core_inputs(x, W, core, em):
    sl = slice(core * CL, (core + 1) * CL)
    xs = np.ascontiguousarray(x[:, :, sl])  # (B, I, CL)
    ws = np.ascontiguousarray(W[0, sl])     # (CL, U, S, I)
    xt = xs.transpose(2, 1, 0).reshape(T, 128, B)
    xt = np.ascontiguousarray(xt.transpose(1, 0, 2)).reshape(128, T * B)
    xf = xs.transpose(0, 2, 1).reshape(2, 128, KCI)
    xf = np.ascontiguousarray(xf.transpose(1, 0, 2)).reshape(128, 2 * KCI)
    wm = ws.transpose(0, 3, 2, 1).reshape(T, 128, US)  # (c,i,s,u), u innermost
    wm = np.ascontiguousarray(wm.transpose(1, 0, 2)).reshape(128, T * US)
    return {
        "xt": xt.astype(np.float16),
        "xf": xf.astype(np.float16),
        "wm": wm.astype(np.float16),
        "em": em,
    }


def prep_in_maps(x, W):
    x = np.asarray(x, dtype=np.float32)
    W = np.asarray(W, dtype=np.float32)
    em = (np.kron(np.eye(8, dtype=np.float32), np.ones((16, 16), np.float32))
          / float(B)).astype(np.float16)
    return [_prep_core_inputs(x, W, core, em) for core in range(NCORES)]


def postprocess(results):
    """Assemble per-core ReduceScatter shards (16 partition rows each) into
    the full [128, 640] (col = bc*320 + s*10 + u), then -> (B, U, S, 1)."""
    pr = 128 // NCORES
    full = np.concatenate(
        [np.asarray(results[r]["out"], np.float32)[0:pr] for r in range(NCORES)],
        axis=0,
    )
    v = full.reshape(128, 2, S, NUM_U).transpose(1, 0, 3, 2)  # (bc,p,u,s)
    return np.ascontiguousarray(v.reshape(B, NUM_U, S)[..., None])


def get_program():
    if "nc" not in _CACHE:
        nc = _build_program()
        nc.finalize()  # runs Bacc.compile(): reg alloc + sync-wait legalization
        _CACHE["nc"] = nc
    return _CACHE["nc"]


def kernel(x, W):
    from concourse.bass_utils import run_bass_kernel_spmd

    nc = get_program()
    in_maps = prep_in_maps(x, W)
    res = run_bass_kernel_spmd(nc, in_maps, list(range(NCORES)))
    return postprocess(res.results)

